# revision 1
# baseline (speedup 1.0000x reference)
"""DeepSeek-style MLA decode attention (batch=8, 128 heads, cache 512) on 8 NeuronCores.

Sharding: tensor-parallel over heads (16 heads/core).
 - q LoRA path sharded over the rank dim (Wq_down cols / Wq_up rows); partial
   q summed+scattered to head owners with a ReduceScatter.
 - Wkv_down replicated (c_kv computed fully on every core).
 - k_cache passed host-pretransposed as [h, b, d, keys]; v_cache as [h, b, keys, d].
 - o_proj input rows sharded by head; partial outputs ReduceScattered over the
   batch dim (core b returns batch b's final row).

Note: the reference's "new token" softmax is over a length-1 axis (== 1.0), so
k_new/Wk_up are dead and the new-token contribution is simply + v_new.
"""

import numpy as np

import concourse.bass as bass
import concourse.mybir as mybir
import concourse.tile as tile
from concourse import bacc
from concourse import bass_utils
from concourse.masks import make_identity

NC_ = 8                      # cores
B = 8                        # batch
H = 128                      # total heads
HP = H // NC_                # 16 heads per core
D = 128                      # head dim
L = 512                      # cache len
HID = 7168
QL = 1536
QLP = QL // NC_              # 192
KVL = 512
NH = HP * D                  # 2048 per-core head cols
SCALE = 1.0 / float(np.sqrt(D))
F32 = mybir.dt.float32
# float32r: single-pass fp32 matmul mode (1 cycle/row at N>=256 vs 4 for
# two-pass fp32). Slightly reduced multiply precision; flip off if the
# accuracy gate complains.
USE_F32R = True


F32R = mybir.dt.float32r
MMD = F32R if USE_F32R else F32  # dtype for matmul-operand tiles


def _rb(ap):
    """Bitcast a DRAM f32 source AP for DMA into a float32r tile."""
    return ap.bitcast(F32R) if USE_F32R else ap


def build_nc():
    nc = bacc.Bacc(
        "TRN2",
        target_bir_lowering=False,
        debug=False,
        enable_asserts=True,
        num_devices=NC_,
    )
    xt = nc.dram_tensor("xt", [HID, B], F32, kind="ExternalInput").ap()
    w_down = nc.dram_tensor("w_down", [HID, QLP + KVL], F32, kind="ExternalInput").ap()
    wq_up = nc.dram_tensor("wq_up", [QLP, H * D], F32, kind="ExternalInput").ap()
    wv_up = nc.dram_tensor("wv_up", [KVL, NH], F32, kind="ExternalInput").ap()
    kt = nc.dram_tensor("kt", [32, 128, 2048], F32, kind="ExternalInput").ap()
    v = nc.dram_tensor("v", [32, 128, 2048], F32, kind="ExternalInput").ap()
    wo = nc.dram_tensor("wo", [NH, HID], F32, kind="ExternalInput").ap()
    o = nc.dram_tensor("o", [1, HID], F32, kind="ExternalOutput").ap()

    rg = [list(range(NC_))]

    with tile.TileContext(nc) as tc:
        with (
            tc.tile_pool(name="const", bufs=1) as constp,
            tc.tile_pool(name="sbuf", bufs=1) as sb,
            tc.tile_pool(name="stage", bufs=2) as stg,
            tc.tile_pool(name="wdown", bufs=3) as wdp,
            tc.tile_pool(name="wqup", bufs=2) as wqp,
            tc.tile_pool(name="ktp", bufs=3) as ktp,
            tc.tile_pool(name="vp", bufs=3) as vp,
            tc.tile_pool(name="wop", bufs=3) as wop,
            tc.tile_pool(name="psbank", bufs=6, space="PSUM") as psbank,
            tc.tile_pool(name="pstr", bufs=2, space="PSUM") as pstr,
            tc.tile_pool(name="dram", bufs=1, space="DRAM") as dram,
        ):
            ident = constp.tile([128, 128], F32)
            make_identity(nc, ident[:])
            id8 = ident[0:8, 0:8]
            # uint8 one-hot columns for CopyPredicated masks (must be int dtype)
            identu8 = constp.tile([128, 128], mybir.dt.uint8, tag="identu8")
            nc.vector.tensor_copy(identu8[:], ident[:])

            # ---------------- q path: cdown = x @ [Wq_down_c | Wkv_down] ----------------
            xt_sb = constp.tile([128, 56 * B], MMD, tag="xt")
            nc.sync.dma_start(
                out=xt_sb[:].rearrange("p (c b) -> p c b", c=56),
                in_=_rb(xt).rearrange("(c p) b -> p c b", p=128),
            )
            ps_cd0 = psbank.tile([8, 512], F32, tag="bank")
            ps_cd1 = psbank.tile([8, 512], F32, tag="bank")
            for i in range(56):
                wd_t = wdp.tile([128, QLP + KVL], MMD, tag="wd")
                nc.sync.dma_start(out=wd_t[:], in_=_rb(w_down)[i * 128:(i + 1) * 128, :])
                lhs = xt_sb[:, i * B:(i + 1) * B]
                nc.tensor.matmul(
                    ps_cd0[:8, 0:512], (lhs), (wd_t[:, 0:512]),
                    start=(i == 0), stop=(i == 55),
                )
                nc.tensor.matmul(
                    ps_cd1[:8, 0:192], lhs, wd_t[:, 512:704],
                    start=(i == 0), stop=(i == 55),
                )
            cdown = sb.tile([8, QLP + KVL], F32, tag="cdown")
            nc.vector.tensor_copy(cdown[:, 0:512], ps_cd0[:8, 0:512])
            nc.vector.tensor_copy(cdown[:, 512:704], ps_cd1[:8, 0:192])

            # transposes: cqT [192, 8] (2 chunks), ckvT [512dims -> 4 chunks of [128, 8]]
            ps_cqT = pstr.tile([128, 128], F32, tag="tr")
            nc.tensor.transpose(ps_cqT[0:128, 0:8], cdown[:, 0:128], id8)
            nc.tensor.transpose(ps_cqT[0:64, 8:16], cdown[:, 128:192], id8)
            ps_ckvT = pstr.tile([128, 128], F32, tag="tr")
            for j in range(4):
                nc.tensor.transpose(
                    ps_ckvT[0:128, j * 8:(j + 1) * 8],
                    cdown[:, QLP + j * 128:QLP + (j + 1) * 128],
                    id8,
                )
            cqT = sb.tile([128, 16], MMD, tag="cqT")
            nc.vector.tensor_copy(cqT[:, 0:8], ps_cqT[:, 0:8])
            nc.vector.tensor_copy(cqT[0:64, 8:16], ps_cqT[0:64, 8:16])
            ckvT = sb.tile([128, 32], MMD, tag="ckvT")
            nc.vector.tensor_copy(ckvT[:, 0:32], ps_ckvT[:, 0:32])

            # ---------------- q_part = cq @ Wq_up_c  (8, 16384) ----------------
            # The 8 col-groups of 2048 are exactly the per-core head groups g;
            # store each to q_bounce[g] for the ReduceScatter.
            q_bounce = dram.tile([NC_ * B, NH], F32, tag="qb")
            for g in range(NC_):
                wqA = wqp.tile([128, 2048], MMD, tag="wqA")
                nc.sync.dma_start(
                    out=wqA[:], in_=_rb(wq_up)[0:128, g * 2048:(g + 1) * 2048]
                )
                wqB = wqp.tile([64, 2048], MMD, tag="wqB")
                nc.sync.dma_start(
                    out=wqB[:], in_=_rb(wq_up)[128:192, g * 2048:(g + 1) * 2048]
                )
                qstage = stg.tile([8, NH], F32, tag="qstage")
                for j in range(4):
                    ps_q = psbank.tile([8, 512], F32, tag="bank")
                    nc.tensor.matmul(
                        ps_q[:8, :], (cqT[:, 0:8]), (wqA[:, j * 512:(j + 1) * 512]),
                        start=True, stop=False,
                    )
                    nc.tensor.matmul(
                        ps_q[:8, :], (cqT[0:64, 8:16]), (wqB[:, j * 512:(j + 1) * 512]),
                        start=False, stop=True,
                    )
                    nc.vector.tensor_copy(
                        qstage[:, j * 512:(j + 1) * 512], ps_q[:8, :]
                    )
                nc.sync.dma_start(
                    out=q_bounce[g * B:(g + 1) * B, :], in_=qstage[:]
                )
            q_rs = dram.tile([B, NH], F32, tag="qrs")
            nc.gpsimd.collective_compute(
                "ReduceScatter",
                mybir.AluOpType.add,
                replica_groups=rg,
                ins=[q_bounce.opt()],
                outs=[q_rs.opt()],
            )
            qown = sb.tile([8, NH], F32, tag="qown")
            nc.sync.dma_start(out=qown[:], in_=q_rs[:])

            # ---------------- v_new = ckv @ Wv_up_c  (8, 2048) ----------------
            wvup = constp.tile([128, 4 * NH], MMD, tag="wvup")
            nc.sync.dma_start(
                out=wvup[:].rearrange("p (c n) -> p c n", c=4),
                in_=_rb(wv_up).rearrange("(c p) n -> p c n", p=128),
            )
            vnew = sb.tile([8, NH], F32, tag="vnew")
            for j in range(4):
                ps_v = psbank.tile([8, 512], F32, tag="bank")
                for cc in range(4):
                    nc.tensor.matmul(
                        ps_v[:8, :],
                        (ckvT[:, cc * 8:(cc + 1) * 8]),
                        (wvup[:, cc * NH + j * 512:cc * NH + (j + 1) * 512]),
                        start=(cc == 0), stop=(cc == 3),
                    )
                nc.vector.tensor_copy(vnew[:, j * 512:(j + 1) * 512], ps_v[:8, :])

            # qT [128 d, hb] via 16 transposes
            ps_qT = pstr.tile([128, 128], F32, tag="tr")
            for h in range(HP):
                nc.tensor.transpose(
                    ps_qT[0:128, h * 8:(h + 1) * 8],
                    qown[:, h * D:(h + 1) * D],
                    id8,
                )
            qT = sb.tile([128, 128], MMD, tag="qT")
            nc.vector.tensor_copy(qT[:], ps_qT[:])

            # ---------------- phase A: scores over k cache ----------------
            # lhsT = qT (all 128 hb) stationary; rhs = kT tile (moving, N=512).
            # Out row hb of each full-bank product is the valid score row;
            # extract it with a partition-aligned copy.
            scores = sb.tile([128, 512], F32, tag="scores")
            for t in range(32):
                kt_t = ktp.tile([128, 2048], MMD, tag="kt")
                nc.sync.dma_start(out=kt_t[:], in_=_rb(kt)[t])
                for u in range(4):
                    hb = 4 * t + u
                    ps_s = psbank.tile([128, 512], F32, tag="bank")
                    nc.tensor.matmul(
                        ps_s[:],
                        (qT[:]),
                        (kt_t[:, u * 512:(u + 1) * 512]),
                        start=True, stop=True,
                    )
                    # write only row hb (engines can't address partition hb
                    # directly: start partition must be 0/32/64/96)
                    nc.vector.copy_predicated(
                        scores[:],
                        identu8[:, hb:hb + 1].broadcast_to((128, 512)),
                        ps_s[:],
                    )

            probs = sb.tile([128, 512], F32, tag="probs")
            denom = sb.tile([128, 1], F32, tag="denom")
            nc.scalar.activation(
                probs[:], scores[:], mybir.ActivationFunctionType.Exp,
                scale=SCALE, accum_out=denom[:],
            )
            recip = sb.tile([128, 1], F32, tag="recip")
            nc.vector.reciprocal(recip[:], denom[:])
            probsn = sb.tile([128, 512], F32, tag="probsn")
            nc.vector.tensor_scalar_mul(probsn[:], probs[:], recip[:])

            ps_pT = psbank.tile([128, 512], F32, tag="bank")
            for cc in range(4):
                nc.tensor.transpose(
                    ps_pT[:, cc * 128:(cc + 1) * 128],
                    probsn[:, cc * 128:(cc + 1) * 128],
                    ident[:],
                )
            probsT = sb.tile([128, 512], MMD, tag="probsT")
            nc.vector.tensor_copy(probsT[:], ps_pT[:])

            # ---------------- phase B: attn rows = probs @ V ----------------
            # Per group of 4 hb: lhsT = probsT chunk c (all hb), rhs packs the
            # 4 hb's V chunk c side by side; accumulate over c, then extract
            # row 4g+u from column block u.
            attn = sb.tile([128, 128], F32, tag="attn")
            for g in range(32):
                v_t = vp.tile([128, 2048], MMD, tag="v")
                nc.sync.dma_start(out=v_t[:], in_=_rb(v)[g])
                ps_a = psbank.tile([128, 512], F32, tag="bank")
                for cc in range(4):
                    nc.tensor.matmul(
                        ps_a[:],
                        (probsT[:, cc * 128:(cc + 1) * 128]),
                        (v_t[:, cc * 512:(cc + 1) * 512]),
                        start=(cc == 0), stop=(cc == 3),
                    )
                for u in range(4):
                    hb = 4 * g + u
                    nc.vector.copy_predicated(
                        attn[:],
                        identu8[:, hb:hb + 1].broadcast_to((128, 128)),
                        ps_a[:, u * 128:(u + 1) * 128],
                    )

            # attnT = attn^T + v_new^T
            ps_vT = pstr.tile([128, 128], F32, tag="tr")
            for h in range(HP):
                nc.tensor.transpose(
                    ps_vT[0:128, h * 8:(h + 1) * 8],
                    vnew[:, h * D:(h + 1) * D],
                    id8,
                )
            vnewT = sb.tile([128, 128], F32, tag="vnewT")
            nc.vector.tensor_copy(vnewT[:], ps_vT[:])
            ps_aT = pstr.tile([128, 128], F32, tag="tr")
            nc.tensor.transpose(ps_aT[:], attn[:], ident[:])
            attnT = sb.tile([128, 128], MMD, tag="attnT")
            nc.vector.tensor_add(attnT[:], ps_aT[:], vnewT[:])

            # ---------------- phase C: o_part = attn^T @ Wo_c ----------------
            # Rounds of up to 6 n-chunks so the accumulators fit in the bank
            # pool; Wo streams as per-head row blocks (large contiguous runs).
            o_bounce = dram.tile([B, HID], F32, tag="ob")
            for n0, n1 in ((0, 6), (6, 12), (12, 14)):
                nn = n1 - n0
                ps_os = [
                    psbank.tile([8, 512], F32, tag="bank", name=f"ps_o{n0}_{i}")
                    for i in range(nn)
                ]
                for h in range(HP):
                    wo_t = wop.tile([128, 3072], MMD, tag="wo")
                    nc.sync.dma_start(
                        out=wo_t[:, 0:nn * 512],
                        in_=_rb(wo)[h * D:(h + 1) * D, n0 * 512:n1 * 512],
                    )
                    for i in range(nn):
                        nc.tensor.matmul(
                            ps_os[i][:8, :],
                            (attnT[:, h * 8:(h + 1) * 8]),
                            (wo_t[:, i * 512:(i + 1) * 512]),
                            start=(h == 0), stop=(h == HP - 1),
                        )
                for i in range(nn):
                    ostage = stg.tile([8, 512], F32, tag="ostage")
                    nc.vector.tensor_copy(ostage[:], ps_os[i][:8, :])
                    nc.sync.dma_start(
                        out=o_bounce[:, (n0 + i) * 512:(n0 + i + 1) * 512],
                        in_=ostage[:],
                    )

            o_rs = dram.tile([1, HID], F32, tag="ors")
            nc.gpsimd.collective_compute(
                "ReduceScatter",
                mybir.AluOpType.add,
                replica_groups=rg,
                ins=[o_bounce.opt()],
                outs=[o_rs.opt()],
            )
            nc.sync.dma_start(out=o[:], in_=o_rs[:])

    nc.compile()
    return nc


_NC_CACHE = None


def _get_nc():
    global _NC_CACHE
    if _NC_CACHE is None:
        _NC_CACHE = build_nc()
    return _NC_CACHE


def make_in_maps(x, k_cache, v_cache, Wq_down, Wq_up, Wkv_down, Wv_up, Wo):
    x2 = np.ascontiguousarray(np.asarray(x, dtype=np.float32).reshape(B, HID).T)
    in_maps = []
    for c in range(NC_):
        hs = slice(c * HP, (c + 1) * HP)
        w_down_c = np.ascontiguousarray(
            np.concatenate(
                [Wq_down[:, c * QLP:(c + 1) * QLP], Wkv_down], axis=1
            ).astype(np.float32)
        )
        wq_up_c = np.ascontiguousarray(Wq_up[c * QLP:(c + 1) * QLP, :], dtype=np.float32)
        wv_up_c = np.ascontiguousarray(
            Wv_up[:, c * HP * D:(c + 1) * HP * D], dtype=np.float32
        )
        wo_c = np.ascontiguousarray(
            Wo[c * HP * D:(c + 1) * HP * D, :], dtype=np.float32
        )
        # kt tile g holds hb=4g..4g+4 as [128 d, (t, k)]; hb=(h, b) row-major
        kt_c = np.ascontiguousarray(
            np.asarray(k_cache, dtype=np.float32)[:, hs]
            .transpose(1, 0, 3, 2)          # (16, 8, 128, 512) [h, b, d, k]
            .reshape(32, 4, 128, 512)       # [g, t, d, k]
            .transpose(0, 2, 1, 3)          # [g, d, t, k]
            .reshape(32, 128, 2048)
        )
        # v tile g holds hb=4g..4g+4 as [128 k, (c, t, d)]
        v_c = np.ascontiguousarray(
            np.asarray(v_cache, dtype=np.float32)[:, hs]
            .transpose(1, 0, 2, 3)          # (16, 8, 512, 128) [h, b, l, d]
            .reshape(32, 4, 4, 128, 128)    # [g, t, c, k, d]
            .transpose(0, 3, 2, 1, 4)       # [g, k, c, t, d]
            .reshape(32, 128, 2048)
        )
        in_maps.append(
            {
                "xt": x2,
                "w_down": w_down_c,
                "wq_up": wq_up_c,
                "wv_up": wv_up_c,
                "kt": kt_c,
                "v": v_c,
                "wo": wo_c,
            }
        )
    return in_maps


def kernel(x, k_cache, v_cache, Wq_down, Wq_up, Wkv_down, Wk_up, Wv_up, Wo, **_):
    x = np.asarray(x, dtype=np.float32)
    in_maps = make_in_maps(
        x, np.asarray(k_cache), np.asarray(v_cache),
        np.asarray(Wq_down, dtype=np.float32), np.asarray(Wq_up, dtype=np.float32),
        np.asarray(Wkv_down, dtype=np.float32), np.asarray(Wv_up, dtype=np.float32),
        np.asarray(Wo, dtype=np.float32),
    )
    nc = _get_nc()
    res = bass_utils.run_bass_kernel_spmd(nc, in_maps, core_ids=list(range(NC_)))
    out = np.stack([res.results[b]["o"] for b in range(B)], axis=0)  # (8, 1, 7168)
    return np.ascontiguousarray(out, dtype=np.float32)



# revision 13
# speedup vs baseline: 2.4132x; 2.4132x over previous
"""DeepSeek-style MLA decode attention (batch=8, 128 heads, cache 512) on 8 NeuronCores.

Sharding: tensor-parallel over heads (16 heads/core).
 - down-projection [Wq_down | Wkv_down] sharded by OUTPUT column (256 cols/core);
   each core computes its cdown column slice (transposed) and a tiny AllGather
   (256x8 f32) replicates the full cdownT to every core.
 - Wq_up / Wv_up / k_cache / v_cache / Wo sharded by head.
 - o_proj partials ReduceScattered over batch (core b returns batch b's row).

All big streams are cast on the host: weights bf16, wq_up + kv caches fp8(e4m3).
Every matmul keeps the large tensor as the STATIONARY operand (it has to pass
through the PE array exactly once either way) and streams the 8-wide batch as
the moving operand, so every intermediate comes out feature-major ("pre-
transposed") and chains straight into the next matmul: no transposes and no
per-row extraction anywhere.

Note: the reference's "new token" softmax is over a length-1 axis (== 1.0), so
k_new/Wk_up are dead and the new-token contribution is simply + v_new.
"""

import numpy as np
import ml_dtypes

import concourse.bass as bass
import concourse.mybir as mybir
import concourse.tile as tile
from concourse import bacc
from concourse import bass_utils

NC_ = 8                      # cores
B = 8                        # batch
H = 128                      # total heads
HP = H // NC_                # 16 heads per core
D = 128                      # head dim
L = 512                      # cache len
HID = 7168
QL = 1536
KVL = 512
COLS = QL + KVL              # 2048 down-proj output cols
COLP = COLS // NC_           # 256 cols per core
NH = HP * D                  # 2048 per-core head cols
SCALE = 1.0 / float(np.sqrt(D))

F32 = mybir.dt.float32
BF = mybir.dt.bfloat16

# fp8 (e4m3) for the attention streams; weights are scaled on the host to sit
# in e4m3's normal range and the inverse scale is folded into the softmax /
# copy scales below.
USE_FP8_KV = True            # kt, v, qT, probsT
USE_FP8_WQ = True            # wq_up, cqT
KVDT = mybir.dt.float8e4 if USE_FP8_KV else BF
WQDT = mybir.dt.float8e4 if USE_FP8_WQ else BF
WQ_SCALE = 16.0 if USE_FP8_WQ else 1.0   # host multiplies Wq_up by this
QT_SCALE = (1.0 / 32.0) if USE_FP8_KV else 1.0  # applied when casting qT
# scoresT = (WQ_SCALE * QT_SCALE) * true_score
EXP_SCALE = SCALE / (WQ_SCALE * QT_SCALE)

NP_BF16 = ml_dtypes.bfloat16
NP_FP8 = ml_dtypes.float8_e4m3


def build_nc():
    nc = bacc.Bacc(
        "TRN2",
        target_bir_lowering=False,
        debug=False,
        enable_asserts=True,
        num_devices=NC_,
    )
    xt = nc.dram_tensor("xt", [128, 56 * B], BF, kind="ExternalInput").ap()
    w_down = nc.dram_tensor("w_down", [128, 56 * COLP], BF, kind="ExternalInput").ap()
    wq_up = nc.dram_tensor("wq_up", [12, 128, NH], WQDT, kind="ExternalInput").ap()
    wv_up = nc.dram_tensor("wv_up", [4, 128, NH], BF, kind="ExternalInput").ap()
    kt = nc.dram_tensor("kt", [32, 128, 2048], KVDT, kind="ExternalInput").ap()
    v = nc.dram_tensor("v", [32, 128, 2048], KVDT, kind="ExternalInput").ap()
    wo = nc.dram_tensor("wo", [NH, HID], BF, kind="ExternalInput").ap()
    o = nc.dram_tensor("o", [1, HID], F32, kind="ExternalOutput").ap()

    rg = [list(range(NC_))]

    with tile.TileContext(nc) as tc:
        with (
            tc.tile_pool(name="const", bufs=1) as constp,
            tc.tile_pool(name="sbuf", bufs=1) as sb,
            tc.tile_pool(name="stage", bufs=2) as stg,
            tc.tile_pool(name="wqp", bufs=12) as wqp,
            tc.tile_pool(name="wvp", bufs=4) as wvp,
            tc.tile_pool(name="ktp", bufs=8) as ktp,
            tc.tile_pool(name="vp", bufs=8) as vp,
            tc.tile_pool(name="wop", bufs=4) as wop,
            tc.tile_pool(name="ps6", bufs=6, space="PSUM") as ps6,
            tc.tile_pool(name="dram", bufs=1, space="DRAM") as dram,
        ):
            # ---------------- load x and the down-proj column shard ----------------
            xt_sb = constp.tile([128, 56 * B], BF, tag="xt")
            nc.sync.dma_start(out=xt_sb[:], in_=xt[:])
            wd_sb = constp.tile([128, 56 * COLP], BF, tag="wd")
            # 14 column-split DMAs so the shard lands on many rings in parallel
            for s in range(14):
                c0 = s * 4 * COLP
                c1 = (s + 1) * 4 * COLP
                nc.sync.dma_start(out=wd_sb[:, c0:c1], in_=w_down[:, c0:c1])

            # ---------------- cdownT column slice: [COLP, B] ----------------
            # cdT[n, b] = sum_hid Wcat[hid, c*COLP+n] * x[b, hid]
            ps_cd = ps6.tile([128, 512], F32, tag="bank", name="ps_cd")
            for half in range(2):
                for i in range(56):
                    nc.tensor.matmul(
                        ps_cd[0:128, half * B:(half + 1) * B],
                        wd_sb[:, i * COLP + half * 128:i * COLP + (half + 1) * 128],
                        xt_sb[:, i * B:(i + 1) * B],
                        start=(i == 0), stop=(i == 55),
                    )
            cd_sb = sb.tile([128, 16], F32, tag="cd")
            nc.vector.tensor_copy(cd_sb[:], ps_cd[0:128, 0:16])

            cd_bounce = dram.tile([COLP, B], F32, tag="cdb")
            nc.sync.dma_start(
                out=cd_bounce[:].rearrange("(h p) b -> p h b", p=128),
                in_=cd_sb[:].rearrange("p (h b) -> p h b", h=2),
            )
            cd_gathered = dram.tile([COLS, B], F32, tag="cdg")
            nc.gpsimd.collective_compute(
                "AllGather",
                mybir.AluOpType.bypass,
                replica_groups=rg,
                ins=[cd_bounce.opt()],
                outs=[cd_gathered.opt()],
            )
            # cdT_sb[p, i*8+b] = cdown col (i*128+p), batch b
            cdT_sb = sb.tile([128, 128], F32, tag="cdT")
            nc.sync.dma_start(
                out=cdT_sb[:].rearrange("p (i b) -> p i b", i=16),
                in_=cd_gathered[:].rearrange("(i p) b -> p i b", p=128),
            )
            # chunks 0..11 = q rank (1536), 12..15 = kv rank (512)
            cqT = sb.tile([128, 96], WQDT, tag="cqT")
            nc.vector.tensor_copy(cqT[:], cdT_sb[:, 0:96])
            ckvT = sb.tile([128, 32], BF, tag="ckvT")
            nc.vector.tensor_copy(ckvT[:], cdT_sb[:, 96:128])

            # ---------------- qT = (cq @ Wq_up_c)^T per head: [128 d, 16h*8b] ----------------
            ps_qT = ps6.tile([128, 512], F32, tag="bank", name="ps_qT")
            wq_tiles = []
            for j in range(12):
                wq_t = wqp.tile([128, NH], WQDT, tag="wq")
                nc.sync.dma_start(out=wq_t[:], in_=wq_up[j])
                wq_tiles.append(wq_t)
            for h in range(HP):
                for j in range(12):
                    nc.tensor.matmul(
                        ps_qT[0:128, h * B:(h + 1) * B],
                        wq_tiles[j][:, h * D:(h + 1) * D],
                        cqT[:, j * B:(j + 1) * B],
                        start=(j == 0), stop=(j == 11),
                    )
            qT = sb.tile([128, 128], KVDT, tag="qT")
            nc.scalar.activation(
                qT[:], ps_qT[0:128, 0:128],
                mybir.ActivationFunctionType.Copy, scale=QT_SCALE,
            )

            # ---------------- v_newT per head: [128 d, 16h*8b] ----------------
            ps_vn = ps6.tile([128, 512], F32, tag="bank", name="ps_vn")
            wv_tiles = []
            for j in range(4):
                wv_t = wvp.tile([128, NH], BF, tag="wv")
                nc.sync.dma_start(out=wv_t[:], in_=wv_up[j])
                wv_tiles.append(wv_t)
            for h in range(HP):
                for j in range(4):
                    nc.tensor.matmul(
                        ps_vn[0:128, h * B:(h + 1) * B],
                        wv_tiles[j][:, h * D:(h + 1) * D],
                        ckvT[:, j * B:(j + 1) * B],
                        start=(j == 0), stop=(j == 3),
                    )

            # ---------------- phase A: scoresT [128 k, 4j * 128 hb] ----------------
            # kt tile g holds hb=4g..4g+3 as [128 d, (u, l)]; stationary slice
            # [128 d, 128 k] per (hb, j), moving = qT column hb.
            ps_sc = ps6.tile([128, 512], F32, tag="bank", name="ps_sc")
            for g in range(32):
                kt_t = ktp.tile([128, 2048], KVDT, tag="kt")
                nc.sync.dma_start(out=kt_t[:], in_=kt[g])
                for u in range(4):
                    hb = 4 * g + u
                    for j in range(4):
                        nc.tensor.matmul(
                            ps_sc[0:128, j * 128 + hb:j * 128 + hb + 1],
                            kt_t[:, u * 512 + j * 128:u * 512 + (j + 1) * 128],
                            qT[:, hb:hb + 1],
                            start=True, stop=True,
                        )

            # softmax pieces: probsT = exp(EXP_SCALE * scoresT) (unnormalized),
            # denom via ones-matmul, reciprocal broadcast to [128, hb].
            # bias shifts exp into fp8 range (max |score*scale| ~ 6); softmax
            # is shift-invariant so the denominator cancels it exactly.
            probsT = sb.tile([128, 512], KVDT, tag="probsT")
            exp_bias = constp.tile([128, 1], F32, tag="exp_bias")
            nc.vector.memset(exp_bias[:], -2.5)
            nc.scalar.activation(
                probsT[:], ps_sc[0:128, 0:512],
                mybir.ActivationFunctionType.Exp, scale=EXP_SCALE, bias=exp_bias[:],
            )
            ones_kv = constp.tile([128, 1], KVDT, tag="ones_kv")
            nc.vector.memset(ones_kv[:], 1.0)
            ps_dn = ps6.tile([1, 128], F32, tag="bank", name="ps_dn")
            for j in range(4):
                nc.tensor.matmul(
                    ps_dn[0:1, 0:128],
                    ones_kv[:],
                    probsT[:, j * 128:(j + 1) * 128],
                    start=(j == 0), stop=(j == 3),
                )
            recip = sb.tile([1, 128], F32, tag="recip")
            nc.vector.reciprocal(recip[:], ps_dn[0:1, 0:128])
            ones_f32 = constp.tile([1, 128], F32, tag="ones_f32")
            nc.vector.memset(ones_f32[:], 1.0)
            ps_rb = ps6.tile([128, 128], F32, tag="bank", name="ps_rb")
            nc.tensor.matmul(
                ps_rb[0:128, 0:128], ones_f32[:], recip[:], start=True, stop=True,
            )
            rb_sb = sb.tile([128, 128], F32, tag="rb_sb")
            nc.vector.tensor_copy(rb_sb[:], ps_rb[0:128, 0:128])

            # ---------------- phase B: cacheT [128 d, 128 hb] ----------------
            # v tile g holds hb=4g..4g+3 as [128 kk, (u, j, d)]; stationary
            # slice [128 kk, 128 d] per (hb, j), moving = probsT column.
            ps_ca = ps6.tile([128, 128], F32, tag="bank", name="ps_ca")
            for g in range(32):
                v_t = vp.tile([128, 2048], KVDT, tag="v")
                nc.sync.dma_start(out=v_t[:], in_=v[g])
                for u in range(4):
                    hb = 4 * g + u
                    for j in range(4):
                        nc.tensor.matmul(
                            ps_ca[0:128, hb:hb + 1],
                            v_t[:, (u * 4 + j) * 128:(u * 4 + j + 1) * 128],
                            probsT[:, j * 128 + hb:j * 128 + hb + 1],
                            start=(j == 0), stop=(j == 3),
                        )

            # attnT = cacheT * recip_bcast + v_newT   [128 d, 128 hb] bf16
            tmp_at = sb.tile([128, 128], F32, tag="tmp_at")
            nc.vector.tensor_mul(tmp_at[:], ps_ca[0:128, 0:128], rb_sb[:])
            attnT = sb.tile([128, 128], BF, tag="attnT")
            nc.vector.tensor_add(attnT[:], tmp_at[:], ps_vn[0:128, 0:128])

            # ---------------- phase C: o_part = attnT.T @ Wo_c ----------------
            o_bounce = dram.tile([B, HID], F32, tag="ob")
            for n0, n1 in ((0, 6), (6, 12), (12, 14)):
                nn = n1 - n0
                ps_os = [
                    ps6.tile([8, 512], F32, tag="bank", name=f"ps_o{n0}_{i}")
                    for i in range(nn)
                ]
                for h in range(HP):
                    wo_t = wop.tile([128, 3072], BF, tag="wo")
                    nc.sync.dma_start(
                        out=wo_t[:, 0:nn * 512],
                        in_=wo[h * D:(h + 1) * D, n0 * 512:n1 * 512],
                    )
                    for i in range(nn):
                        nc.tensor.matmul(
                            ps_os[i][:8, :],
                            attnT[:, h * B:(h + 1) * B],
                            wo_t[:, i * 512:(i + 1) * 512],
                            start=(h == 0), stop=(h == HP - 1),
                        )
                for i in range(nn):
                    ostage = stg.tile([8, 512], F32, tag="ostage")
                    nc.vector.tensor_copy(ostage[:], ps_os[i][:8, :])
                    nc.sync.dma_start(
                        out=o_bounce[:, (n0 + i) * 512:(n0 + i + 1) * 512],
                        in_=ostage[:],
                    )

            o_rs = dram.tile([1, HID], F32, tag="ors")
            nc.gpsimd.collective_compute(
                "ReduceScatter",
                mybir.AluOpType.add,
                replica_groups=rg,
                ins=[o_bounce.opt()],
                outs=[o_rs.opt()],
            )
            nc.sync.dma_start(out=o[:], in_=o_rs[:])

    nc.compile()
    return nc


_NC_CACHE = None


def _get_nc():
    global _NC_CACHE
    if _NC_CACHE is None:
        _NC_CACHE = build_nc()
    return _NC_CACHE


def make_in_maps(x, k_cache, v_cache, Wq_down, Wq_up, Wkv_down, Wv_up, Wo):
    x = np.asarray(x, np.float32).reshape(B, HID)
    k_cache = np.asarray(k_cache, np.float32)
    v_cache = np.asarray(v_cache, np.float32)
    Wq_down = np.asarray(Wq_down, np.float32)
    Wq_up = np.asarray(Wq_up, np.float32)
    Wkv_down = np.asarray(Wkv_down, np.float32)
    Wv_up = np.asarray(Wv_up, np.float32)
    Wo = np.asarray(Wo, np.float32)

    np_kv = NP_FP8 if USE_FP8_KV else NP_BF16
    np_wq = NP_FP8 if USE_FP8_WQ else NP_BF16

    # xt[p, i*8+b] = x[b, i*128+p]
    xt = np.ascontiguousarray(
        x.T.reshape(56, 128, B).transpose(1, 0, 2).reshape(128, 56 * B)
    ).astype(NP_BF16)
    Wcat = np.concatenate([Wq_down, Wkv_down], axis=1)  # [7168, 2048]

    in_maps = []
    for c in range(NC_):
        hs = slice(c * HP, (c + 1) * HP)
        # w_down[p, i*COLP + n] = Wcat[i*128+p, c*COLP+n]
        wd = np.ascontiguousarray(
            Wcat[:, c * COLP:(c + 1) * COLP]
            .reshape(56, 128, COLP).transpose(1, 0, 2).reshape(128, 56 * COLP)
        ).astype(NP_BF16)
        wq = np.ascontiguousarray(
            (Wq_up[:, c * NH:(c + 1) * NH] * WQ_SCALE).reshape(12, 128, NH)
        ).astype(np_wq)
        wv = np.ascontiguousarray(
            Wv_up[:, c * NH:(c + 1) * NH].reshape(4, 128, NH)
        ).astype(NP_BF16)
        wo_c = np.ascontiguousarray(Wo[c * NH:(c + 1) * NH, :]).astype(NP_BF16)
        # kt tile g: [128 d, (u, l)] for hb=4g+u
        kt_c = np.ascontiguousarray(
            k_cache[:, hs]
            .transpose(1, 0, 3, 2)          # [h, b, d, l] (16, 8, 128, 512)
            .reshape(32, 4, 128, 512)       # [g, u, d, l]
            .transpose(0, 2, 1, 3)          # [g, d, u, l]
            .reshape(32, 128, 2048)
        ).astype(np_kv)
        # v tile g: [128 kk, (u, j, d)] for hb=4g+u, l = j*128+kk
        v_c = np.ascontiguousarray(
            v_cache[:, hs]
            .transpose(1, 0, 2, 3)          # [h, b, l, d] (16, 8, 512, 128)
            .reshape(32, 4, 4, 128, 128)    # [g, u, j, kk, d]
            .transpose(0, 3, 1, 2, 4)       # [g, kk, u, j, d]
            .reshape(32, 128, 2048)
        ).astype(np_kv)
        in_maps.append(
            {
                "xt": xt,
                "w_down": wd,
                "wq_up": wq,
                "wv_up": wv,
                "kt": kt_c,
                "v": v_c,
                "wo": wo_c,
            }
        )
    return in_maps


def kernel(x, k_cache, v_cache, Wq_down, Wq_up, Wkv_down, Wk_up, Wv_up, Wo, **_):
    in_maps = make_in_maps(
        x, k_cache, v_cache, Wq_down, Wq_up, Wkv_down, Wv_up, Wo
    )
    nc = _get_nc()
    res = bass_utils.run_bass_kernel_spmd(nc, in_maps, core_ids=list(range(NC_)))
    out = np.stack([res.results[b]["o"] for b in range(B)], axis=0)  # (8, 1, 7168)
    return np.ascontiguousarray(out, dtype=np.float32)


# revision 17
# speedup vs baseline: 2.4974x; 1.0349x over previous
"""DeepSeek-style MLA decode attention (batch=8, 128 heads, cache 512) on 8 NeuronCores.

Sharding: tensor-parallel over heads (16 heads/core).
 - down-projection [Wq_down | Wkv_down] sharded by OUTPUT column (256 cols/core);
   each core computes its cdown column slice (transposed) and a tiny AllGather
   (256x8 f32) replicates the full cdownT to every core.
 - Wq_up / Wv_up / k_cache / v_cache / Wo sharded by head.
 - o_proj partials ReduceScattered over batch (core b returns batch b's row).

All big streams are cast on the host: weights bf16, wq_up + kv caches fp8(e4m3).
Every matmul keeps the large tensor as the STATIONARY operand (it has to pass
through the PE array exactly once either way) and streams the 8-wide batch as
the moving operand, so every intermediate comes out feature-major ("pre-
transposed") and chains straight into the next matmul: no transposes and no
per-row extraction anywhere.

Note: the reference's "new token" softmax is over a length-1 axis (== 1.0), so
k_new/Wk_up are dead and the new-token contribution is simply + v_new.
"""

import numpy as np
import ml_dtypes

import concourse.bass as bass
import concourse.mybir as mybir
import concourse.tile as tile
from concourse import bacc
from concourse import bass_utils

NC_ = 8                      # cores
B = 8                        # batch
H = 128                      # total heads
HP = H // NC_                # 16 heads per core
D = 128                      # head dim
L = 512                      # cache len
HID = 7168
QL = 1536
KVL = 512
COLS = QL + KVL              # 2048 down-proj output cols
COLP = COLS // NC_           # 256 cols per core
NH = HP * D                  # 2048 per-core head cols
SCALE = 1.0 / float(np.sqrt(D))

F32 = mybir.dt.float32
BF = mybir.dt.bfloat16

# fp8 (e4m3) for the attention streams; weights are scaled on the host to sit
# in e4m3's normal range and the inverse scale is folded into the softmax /
# copy scales below.
USE_FP8_KV = True            # kt, v, qT, probsT
USE_FP8_WQ = True            # wq_up, cqT
KVDT = mybir.dt.float8e4 if USE_FP8_KV else BF
WQDT = mybir.dt.float8e4 if USE_FP8_WQ else BF
WQ_SCALE = 16.0 if USE_FP8_WQ else 1.0   # host multiplies Wq_up by this
QT_SCALE = (1.0 / 32.0) if USE_FP8_KV else 1.0  # applied when casting qT
# scoresT = (WQ_SCALE * QT_SCALE) * true_score
EXP_SCALE = SCALE / (WQ_SCALE * QT_SCALE)

NP_BF16 = ml_dtypes.bfloat16
NP_FP8 = ml_dtypes.float8_e4m3


def build_nc():
    nc = bacc.Bacc(
        "TRN2",
        target_bir_lowering=False,
        debug=False,
        enable_asserts=True,
        num_devices=NC_,
    )
    xt = nc.dram_tensor("xt", [128, 56 * B], BF, kind="ExternalInput").ap()
    w_down = nc.dram_tensor("w_down", [128, 56 * COLP], BF, kind="ExternalInput").ap()
    wq_up = nc.dram_tensor("wq_up", [12, 128, NH], WQDT, kind="ExternalInput").ap()
    wv_up = nc.dram_tensor("wv_up", [4, 128, NH], BF, kind="ExternalInput").ap()
    kt = nc.dram_tensor("kt", [32, 128, 2048], KVDT, kind="ExternalInput").ap()
    v = nc.dram_tensor("v", [32, 128, 2048], KVDT, kind="ExternalInput").ap()
    wo = nc.dram_tensor("wo", [NH, HID], BF, kind="ExternalInput").ap()
    o = nc.dram_tensor("o", [1, HID], F32, kind="ExternalOutput").ap()

    rg = [list(range(NC_))]

    with tile.TileContext(nc) as tc:
        with (
            tc.tile_pool(name="const", bufs=1) as constp,
            tc.tile_pool(name="sbuf", bufs=1) as sb,
            tc.tile_pool(name="stage", bufs=2) as stg,
            tc.tile_pool(name="wqp", bufs=12) as wqp,
            tc.tile_pool(name="wvp", bufs=4) as wvp,
            tc.tile_pool(name="ktp", bufs=16) as ktp,
            tc.tile_pool(name="vp", bufs=16) as vp,
            tc.tile_pool(name="wop", bufs=6) as wop,
            tc.tile_pool(name="ps6", bufs=6, space="PSUM") as ps6,
            tc.tile_pool(name="dram", bufs=1, space="DRAM") as dram,
        ):
            # Tiny dummy collective issued first: absorbs the ~35us cross-core
            # barrier / NRT warmup latency of the FIRST collective so the real
            # cdown AllGather (on the critical path) runs on a warm path.
            warm_sb = constp.tile([1, 8], F32, tag="warm")
            nc.vector.memset(warm_sb[:], 0.0)
            warm_in = dram.tile([1, 8], F32, tag="warm_in")
            nc.sync.dma_start(out=warm_in[:], in_=warm_sb[:])
            warm_out = dram.tile([NC_, 8], F32, tag="warm_out")
            nc.gpsimd.collective_compute(
                "AllGather",
                mybir.AluOpType.bypass,
                replica_groups=rg,
                ins=[warm_in.opt()],
                outs=[warm_out.opt()],
            )

            # ---------------- load x and the down-proj column shard ----------------
            xt_sb = constp.tile([128, 56 * B], BF, tag="xt")
            nc.sync.dma_start(out=xt_sb[:], in_=xt[:])
            wd_sb = constp.tile([128, 56 * COLP], BF, tag="wd")
            # 14 column-split DMAs so the shard lands on many rings in parallel
            for s in range(14):
                c0 = s * 4 * COLP
                c1 = (s + 1) * 4 * COLP
                nc.sync.dma_start(out=wd_sb[:, c0:c1], in_=w_down[:, c0:c1])

            # ---------------- cdownT column slice: [COLP, B] ----------------
            # cdT[n, b] = sum_hid Wcat[hid, c*COLP+n] * x[b, hid]
            ps_cd = ps6.tile([128, 512], F32, tag="bank", name="ps_cd")
            cd_sb = sb.tile([128, 16], F32, tag="cd")
            for half in range(2):
                for i in range(56):
                    nc.tensor.matmul(
                        ps_cd[0:128, half * B:(half + 1) * B],
                        wd_sb[:, i * COLP + half * 128:i * COLP + (half + 1) * 128],
                        xt_sb[:, i * B:(i + 1) * B],
                        start=(i == 0), stop=(i == 55),
                    )
                nc.vector.tensor_copy(
                    cd_sb[:, half * B:(half + 1) * B],
                    ps_cd[0:128, half * B:(half + 1) * B],
                )

            cd_bounce = dram.tile([COLP, B], F32, tag="cdb")
            nc.sync.dma_start(
                out=cd_bounce[:].rearrange("(h p) b -> p h b", p=128),
                in_=cd_sb[:].rearrange("p (h b) -> p h b", h=2),
            )
            cd_gathered = dram.tile([COLS, B], F32, tag="cdg")
            nc.gpsimd.collective_compute(
                "AllGather",
                mybir.AluOpType.bypass,
                replica_groups=rg,
                ins=[cd_bounce.opt()],
                outs=[cd_gathered.opt()],
            )
            # cdT_sb[p, i*8+b] = cdown col (i*128+p), batch b
            cdT_sb = sb.tile([128, 128], F32, tag="cdT")
            nc.sync.dma_start(
                out=cdT_sb[:].rearrange("p (i b) -> p i b", i=16),
                in_=cd_gathered[:].rearrange("(i p) b -> p i b", p=128),
            )
            # chunks 0..11 = q rank (1536), 12..15 = kv rank (512)
            cqT = sb.tile([128, 96], WQDT, tag="cqT")
            nc.vector.tensor_copy(cqT[:], cdT_sb[:, 0:96])
            ckvT = sb.tile([128, 32], BF, tag="ckvT")
            nc.vector.tensor_copy(ckvT[:], cdT_sb[:, 96:128])

            # ---------------- qT = (cq @ Wq_up_c)^T per head: [128 d, 16h*8b] ----------------
            ps_qT = ps6.tile([128, 512], F32, tag="bank", name="ps_qT")
            wq_tiles = []
            for j in range(12):
                wq_t = wqp.tile([128, NH], WQDT, tag="wq")
                nc.sync.dma_start(out=wq_t[:], in_=wq_up[j])
                wq_tiles.append(wq_t)
            for h in range(HP):
                for j in range(12):
                    nc.tensor.matmul(
                        ps_qT[0:128, h * B:(h + 1) * B],
                        wq_tiles[j][:, h * D:(h + 1) * D],
                        cqT[:, j * B:(j + 1) * B],
                        start=(j == 0), stop=(j == 11),
                    )
            qT = sb.tile([128, 128], KVDT, tag="qT")
            nc.scalar.activation(
                qT[:], ps_qT[0:128, 0:128],
                mybir.ActivationFunctionType.Copy, scale=QT_SCALE,
            )

            # ---------------- v_newT per head: [128 d, 16h*8b] ----------------
            ps_vn = ps6.tile([128, 512], F32, tag="bank", name="ps_vn")
            wv_tiles = []
            for j in range(4):
                wv_t = wvp.tile([128, NH], BF, tag="wv")
                nc.sync.dma_start(out=wv_t[:], in_=wv_up[j])
                wv_tiles.append(wv_t)
            for h in range(HP):
                for j in range(4):
                    nc.tensor.matmul(
                        ps_vn[0:128, h * B:(h + 1) * B],
                        wv_tiles[j][:, h * D:(h + 1) * D],
                        ckvT[:, j * B:(j + 1) * B],
                        start=(j == 0), stop=(j == 3),
                    )

            # ---------------- phase A: scoresT [128 k, 4j * 128 hb] ----------------
            # kt tile g holds hb=4g..4g+3 as [128 d, (u, l)]; stationary slice
            # [128 d, 128 k] per (hb, j), moving = qT column hb.
            ps_sc = ps6.tile([128, 512], F32, tag="bank", name="ps_sc")
            for g in range(32):
                kt_t = ktp.tile([128, 2048], KVDT, tag="kt")
                nc.sync.dma_start(out=kt_t[:], in_=kt[g])
                for u in range(4):
                    hb = 4 * g + u
                    for j in range(4):
                        nc.tensor.matmul(
                            ps_sc[0:128, j * 128 + hb:j * 128 + hb + 1],
                            kt_t[:, u * 512 + j * 128:u * 512 + (j + 1) * 128],
                            qT[:, hb:hb + 1],
                            start=True, stop=True,
                        )

            # softmax pieces: probsT = exp(EXP_SCALE * scoresT) (unnormalized),
            # denom via ones-matmul, reciprocal broadcast to [128, hb].
            # bias shifts exp into fp8 range (max |score*scale| ~ 6); softmax
            # is shift-invariant so the denominator cancels it exactly.
            probsT = sb.tile([128, 512], KVDT, tag="probsT")
            exp_bias = constp.tile([128, 1], F32, tag="exp_bias")
            nc.vector.memset(exp_bias[:], -2.5)
            nc.scalar.activation(
                probsT[:], ps_sc[0:128, 0:512],
                mybir.ActivationFunctionType.Exp, scale=EXP_SCALE, bias=exp_bias[:],
            )
            ones_kv = constp.tile([128, 1], KVDT, tag="ones_kv")
            nc.vector.memset(ones_kv[:], 1.0)
            ps_dn = ps6.tile([1, 128], F32, tag="bank", name="ps_dn")
            for j in range(4):
                nc.tensor.matmul(
                    ps_dn[0:1, 0:128],
                    ones_kv[:],
                    probsT[:, j * 128:(j + 1) * 128],
                    start=(j == 0), stop=(j == 3),
                )
            recip = sb.tile([1, 128], F32, tag="recip")
            nc.vector.reciprocal(recip[:], ps_dn[0:1, 0:128])
            ones_f32 = constp.tile([1, 128], F32, tag="ones_f32")
            nc.vector.memset(ones_f32[:], 1.0)
            ps_rb = ps6.tile([128, 128], F32, tag="bank", name="ps_rb")
            nc.tensor.matmul(
                ps_rb[0:128, 0:128], ones_f32[:], recip[:], start=True, stop=True,
            )
            rb_sb = sb.tile([128, 128], F32, tag="rb_sb")
            nc.vector.tensor_copy(rb_sb[:], ps_rb[0:128, 0:128])

            # ---------------- phase B: cacheT [128 d, 128 hb] ----------------
            # v tile g holds hb=4g..4g+3 as [128 kk, (u, j, d)]; stationary
            # slice [128 kk, 128 d] per (hb, j), moving = probsT column.
            ps_ca = ps6.tile([128, 128], F32, tag="bank", name="ps_ca")
            for g in range(32):
                v_t = vp.tile([128, 2048], KVDT, tag="v")
                nc.sync.dma_start(out=v_t[:], in_=v[g])
                for u in range(4):
                    hb = 4 * g + u
                    for j in range(4):
                        nc.tensor.matmul(
                            ps_ca[0:128, hb:hb + 1],
                            v_t[:, (u * 4 + j) * 128:(u * 4 + j + 1) * 128],
                            probsT[:, j * 128 + hb:j * 128 + hb + 1],
                            start=(j == 0), stop=(j == 3),
                        )

            # attnT = cacheT * recip_bcast + v_newT   [128 d, 128 hb] bf16
            tmp_at = sb.tile([128, 128], F32, tag="tmp_at")
            nc.vector.tensor_mul(tmp_at[:], ps_ca[0:128, 0:128], rb_sb[:])
            attnT = sb.tile([128, 128], BF, tag="attnT")
            nc.vector.tensor_add(attnT[:], tmp_at[:], ps_vn[0:128, 0:128])

            # ---------------- phase C: o_part = attnT.T @ Wo_c ----------------
            # Output cols split in two halves; each half's ReduceScatter
            # launches as soon as its rounds finish so the collective overlaps
            # the remaining compute.
            o_bounce0 = dram.tile([B, 12 * 512], F32, tag="ob0")
            o_bounce1 = dram.tile([B, HID - 12 * 512], F32, tag="ob1")
            for n0, n1 in ((0, 6), (6, 12), (12, 14)):
                nn = n1 - n0
                ps_os = [
                    ps6.tile([8, 512], F32, tag="bank", name=f"ps_o{n0}_{i}")
                    for i in range(nn)
                ]
                for h in range(HP):
                    wo_t = wop.tile([128, 3072], BF, tag="wo")
                    nc.sync.dma_start(
                        out=wo_t[:, 0:nn * 512],
                        in_=wo[h * D:(h + 1) * D, n0 * 512:n1 * 512],
                    )
                    for i in range(nn):
                        nc.tensor.matmul(
                            ps_os[i][:8, :],
                            attnT[:, h * B:(h + 1) * B],
                            wo_t[:, i * 512:(i + 1) * 512],
                            start=(h == 0), stop=(h == HP - 1),
                        )
                for i in range(nn):
                    ostage = stg.tile([8, 512], F32, tag="ostage")
                    nc.vector.tensor_copy(ostage[:], ps_os[i][:8, :])
                    if n0 + i < 12:
                        nc.sync.dma_start(
                            out=o_bounce0[:, (n0 + i) * 512:(n0 + i + 1) * 512],
                            in_=ostage[:],
                        )
                    else:
                        nc.sync.dma_start(
                            out=o_bounce1[:, (n0 + i - 12) * 512:(n0 + i - 11) * 512],
                            in_=ostage[:],
                        )
                if n1 == 12:
                    o_rs0 = dram.tile([1, 12 * 512], F32, tag="ors0")
                    nc.gpsimd.collective_compute(
                        "ReduceScatter",
                        mybir.AluOpType.add,
                        replica_groups=rg,
                        ins=[o_bounce0.opt()],
                        outs=[o_rs0.opt()],
                    )
                    nc.sync.dma_start(out=o[:, 0:12 * 512], in_=o_rs0[:])

            o_rs1 = dram.tile([1, HID - 12 * 512], F32, tag="ors1")
            nc.gpsimd.collective_compute(
                "ReduceScatter",
                mybir.AluOpType.add,
                replica_groups=rg,
                ins=[o_bounce1.opt()],
                outs=[o_rs1.opt()],
            )
            nc.sync.dma_start(out=o[:, 12 * 512:], in_=o_rs1[:])

    nc.compile()
    return nc


_NC_CACHE = None


def _get_nc():
    global _NC_CACHE
    if _NC_CACHE is None:
        _NC_CACHE = build_nc()
    return _NC_CACHE


def make_in_maps(x, k_cache, v_cache, Wq_down, Wq_up, Wkv_down, Wv_up, Wo):
    x = np.asarray(x, np.float32).reshape(B, HID)
    k_cache = np.asarray(k_cache, np.float32)
    v_cache = np.asarray(v_cache, np.float32)
    Wq_down = np.asarray(Wq_down, np.float32)
    Wq_up = np.asarray(Wq_up, np.float32)
    Wkv_down = np.asarray(Wkv_down, np.float32)
    Wv_up = np.asarray(Wv_up, np.float32)
    Wo = np.asarray(Wo, np.float32)

    np_kv = NP_FP8 if USE_FP8_KV else NP_BF16
    np_wq = NP_FP8 if USE_FP8_WQ else NP_BF16

    # xt[p, i*8+b] = x[b, i*128+p]
    xt = np.ascontiguousarray(
        x.T.reshape(56, 128, B).transpose(1, 0, 2).reshape(128, 56 * B)
    ).astype(NP_BF16)
    Wcat = np.concatenate([Wq_down, Wkv_down], axis=1)  # [7168, 2048]

    in_maps = []
    for c in range(NC_):
        hs = slice(c * HP, (c + 1) * HP)
        # w_down[p, i*COLP + n] = Wcat[i*128+p, c*COLP+n]
        wd = np.ascontiguousarray(
            Wcat[:, c * COLP:(c + 1) * COLP]
            .reshape(56, 128, COLP).transpose(1, 0, 2).reshape(128, 56 * COLP)
        ).astype(NP_BF16)
        wq = np.ascontiguousarray(
            (Wq_up[:, c * NH:(c + 1) * NH] * WQ_SCALE).reshape(12, 128, NH)
        ).astype(np_wq)
        wv = np.ascontiguousarray(
            Wv_up[:, c * NH:(c + 1) * NH].reshape(4, 128, NH)
        ).astype(NP_BF16)
        wo_c = np.ascontiguousarray(Wo[c * NH:(c + 1) * NH, :]).astype(NP_BF16)
        # kt tile g: [128 d, (u, l)] for hb=4g+u
        kt_c = np.ascontiguousarray(
            k_cache[:, hs]
            .transpose(1, 0, 3, 2)          # [h, b, d, l] (16, 8, 128, 512)
            .reshape(32, 4, 128, 512)       # [g, u, d, l]
            .transpose(0, 2, 1, 3)          # [g, d, u, l]
            .reshape(32, 128, 2048)
        ).astype(np_kv)
        # v tile g: [128 kk, (u, j, d)] for hb=4g+u, l = j*128+kk
        v_c = np.ascontiguousarray(
            v_cache[:, hs]
            .transpose(1, 0, 2, 3)          # [h, b, l, d] (16, 8, 512, 128)
            .reshape(32, 4, 4, 128, 128)    # [g, u, j, kk, d]
            .transpose(0, 3, 1, 2, 4)       # [g, kk, u, j, d]
            .reshape(32, 128, 2048)
        ).astype(np_kv)
        in_maps.append(
            {
                "xt": xt,
                "w_down": wd,
                "wq_up": wq,
                "wv_up": wv,
                "kt": kt_c,
                "v": v_c,
                "wo": wo_c,
            }
        )
    return in_maps


def kernel(x, k_cache, v_cache, Wq_down, Wq_up, Wkv_down, Wk_up, Wv_up, Wo, **_):
    in_maps = make_in_maps(
        x, k_cache, v_cache, Wq_down, Wq_up, Wkv_down, Wv_up, Wo
    )
    nc = _get_nc()
    res = bass_utils.run_bass_kernel_spmd(nc, in_maps, core_ids=list(range(NC_)))
    out = np.stack([res.results[b]["o"] for b in range(B)], axis=0)  # (8, 1, 7168)
    return np.ascontiguousarray(out, dtype=np.float32)


# revision 23
# speedup vs baseline: 2.6349x; 1.0550x over previous
"""DeepSeek-style MLA decode attention (batch=8, 128 heads, cache 512) on 8 NeuronCores.

Sharding: tensor-parallel over heads (16 heads/core).
 - down-projection [Wq_down | Wkv_down] sharded by OUTPUT column (256 cols/core);
   each core computes its cdown column slice (transposed) and a tiny AllGather
   (256x8 f32) replicates the full cdownT to every core.
 - Wq_up / Wv_up / k_cache / v_cache / Wo sharded by head.
 - o_proj partials ReduceScattered over batch (core b returns batch b's row).

All big streams are cast on the host: weights bf16, wq_up + kv caches fp8(e4m3).
Every matmul keeps the large tensor as the STATIONARY operand (it has to pass
through the PE array exactly once either way) and streams the 8-wide batch as
the moving operand, so every intermediate comes out feature-major ("pre-
transposed") and chains straight into the next matmul: no transposes and no
per-row extraction anywhere.

Note: the reference's "new token" softmax is over a length-1 axis (== 1.0), so
k_new/Wk_up are dead and the new-token contribution is simply + v_new.
"""

import numpy as np
import ml_dtypes

import concourse.bass as bass
import concourse.mybir as mybir
import concourse.tile as tile
from concourse import bacc
from concourse import bass_utils

NC_ = 8                      # cores
B = 8                        # batch
H = 128                      # total heads
HP = H // NC_                # 16 heads per core
D = 128                      # head dim
L = 512                      # cache len
HID = 7168
QL = 1536
KVL = 512
COLS = QL + KVL              # 2048 down-proj output cols
COLP = COLS // NC_           # 256 cols per core
NH = HP * D                  # 2048 per-core head cols
SCALE = 1.0 / float(np.sqrt(D))

F32 = mybir.dt.float32
BF = mybir.dt.bfloat16

# fp8 (e4m3) for the attention streams; weights are scaled on the host to sit
# in e4m3's normal range and the inverse scale is folded into the softmax /
# copy scales below.
USE_FP8_KV = True            # kt, v, qT, probsT
USE_FP8_WQ = True            # wq_up, cqT
KVDT = mybir.dt.float8e4 if USE_FP8_KV else BF
WQDT = mybir.dt.float8e4 if USE_FP8_WQ else BF
WQ_SCALE = 16.0 if USE_FP8_WQ else 1.0   # host multiplies Wq_up by this
QT_SCALE = (1.0 / 32.0) if USE_FP8_KV else 1.0  # applied when casting qT
# scoresT = (WQ_SCALE * QT_SCALE) * true_score
EXP_SCALE = SCALE / (WQ_SCALE * QT_SCALE)

NP_BF16 = ml_dtypes.bfloat16
NP_FP8 = ml_dtypes.float8_e4m3


def build_nc():
    nc = bacc.Bacc(
        "TRN2",
        target_bir_lowering=False,
        debug=False,
        enable_asserts=True,
        num_devices=NC_,
    )
    xt = nc.dram_tensor("xt", [128, 56 * B], BF, kind="ExternalInput").ap()
    w_down = nc.dram_tensor("w_down", [128, 56 * COLP], BF, kind="ExternalInput").ap()
    wq_up = nc.dram_tensor("wq_up", [12, 128, NH], WQDT, kind="ExternalInput").ap()
    wv_up = nc.dram_tensor("wv_up", [4, 128, NH], BF, kind="ExternalInput").ap()
    kt = nc.dram_tensor("kt", [32, 128, 2048], KVDT, kind="ExternalInput").ap()
    v = nc.dram_tensor("v", [32, 128, 2048], KVDT, kind="ExternalInput").ap()
    wo = nc.dram_tensor("wo", [NH, HID], BF, kind="ExternalInput").ap()
    o = nc.dram_tensor("o", [1, HID], F32, kind="ExternalOutput").ap()

    rg = [list(range(NC_))]

    with tile.TileContext(nc) as tc:
        with (
            tc.tile_pool(name="const", bufs=1) as constp,
            tc.tile_pool(name="sbuf", bufs=1) as sb,
            tc.tile_pool(name="stage", bufs=2) as stg,
            tc.tile_pool(name="wdp", bufs=4) as wdp,
            tc.tile_pool(name="wqp", bufs=12) as wqp,
            tc.tile_pool(name="wvp", bufs=4) as wvp,
            tc.tile_pool(name="ktp", bufs=22) as ktp,
            tc.tile_pool(name="vp", bufs=22) as vp,
            tc.tile_pool(name="wop", bufs=6) as wop,
            tc.tile_pool(name="psA", bufs=2, space="PSUM") as psA,
            tc.tile_pool(name="psB", bufs=6, space="PSUM") as psB,
            tc.tile_pool(name="dram", bufs=1, space="DRAM") as dram,
        ):
            # ---------------- load x and the down-proj column shard ----------------
            xt_sb = constp.tile([128, 56 * B], BF, tag="xt")
            nc.sync.dma_start(out=xt_sb[:], in_=xt[:])

            # ---------------- cdownT column slice: [COLP, B] ----------------
            # cdT[n, b] = sum_hid Wcat[hid, c*COLP+n] * x[b, hid]
            # w_down streams through a small pool: tile s holds i-chunks 4s..4s+3.
            ps_cds = [
                psB.tile([128, 512], F32, tag="bank", name=f"ps_cd{half}")
                for half in range(2)
            ]
            cd_sb = sb.tile([128, 16], F32, tag="cd")
            for s in range(14):
                wd_t = wdp.tile([128, 4 * COLP], BF, tag="wd")
                nc.sync.dma_start(
                    out=wd_t[:], in_=w_down[:, s * 4 * COLP:(s + 1) * 4 * COLP]
                )
                for q in range(4):
                    i = s * 4 + q
                    for half in range(2):
                        nc.tensor.matmul(
                            ps_cds[half][0:128, 0:B],
                            wd_t[:, q * COLP + half * 128:q * COLP + (half + 1) * 128],
                            xt_sb[:, i * B:(i + 1) * B],
                            start=(i == 0), stop=(i == 55),
                        )
            for half in range(2):
                nc.vector.tensor_copy(
                    cd_sb[:, half * B:(half + 1) * B], ps_cds[half][0:128, 0:B]
                )

            cd_bounce = dram.tile([COLP, B], F32, tag="cdb")
            nc.sync.dma_start(
                out=cd_bounce[:].rearrange("(h p) b -> p h b", p=128),
                in_=cd_sb[:].rearrange("p (h b) -> p h b", h=2),
            )
            cd_gathered = dram.tile([COLS, B], F32, tag="cdg")
            nc.gpsimd.collective_compute(
                "AllGather",
                mybir.AluOpType.bypass,
                replica_groups=rg,
                ins=[cd_bounce.opt()],
                outs=[cd_gathered.opt()],
            )
            # cdT_sb[p, i*8+b] = cdown col (i*128+p), batch b
            cdT_sb = sb.tile([128, 128], F32, tag="cdT")
            nc.sync.dma_start(
                out=cdT_sb[:].rearrange("p (i b) -> p i b", i=16),
                in_=cd_gathered[:].rearrange("(i p) b -> p i b", p=128),
            )
            # chunks 0..11 = q rank (1536), 12..15 = kv rank (512)
            cqT = sb.tile([128, 96], WQDT, tag="cqT")
            nc.vector.tensor_copy(cqT[:], cdT_sb[:, 0:96])
            ckvT = sb.tile([128, 32], BF, tag="ckvT")
            nc.vector.tensor_copy(ckvT[:], cdT_sb[:, 96:128])

            # ---------------- qT = (cq @ Wq_up_c)^T per head: [128 d, 16h*8b] ----------------
            ps_qT = psB.tile([128, 512], F32, tag="bank", name="ps_qT")
            wq_tiles = []
            for j in range(12):
                wq_t = wqp.tile([128, NH], WQDT, tag="wq")
                nc.sync.dma_start(out=wq_t[:], in_=wq_up[j])
                wq_tiles.append(wq_t)
            for h in range(HP):
                for j in range(12):
                    nc.tensor.matmul(
                        ps_qT[0:128, h * B:(h + 1) * B],
                        wq_tiles[j][:, h * D:(h + 1) * D],
                        cqT[:, j * B:(j + 1) * B],
                        start=(j == 0), stop=(j == 11),
                    )
            qT = sb.tile([128, 128], KVDT, tag="qT")
            nc.scalar.activation(
                qT[:], ps_qT[0:128, 0:128],
                mybir.ActivationFunctionType.Copy, scale=QT_SCALE,
            )

            # ---------------- v_newT per head: [128 d, 16h*8b] ----------------
            ps_vn = psA.tile([128, 512], F32, tag="bank", name="ps_vn")
            wv_tiles = []
            for j in range(4):
                wv_t = wvp.tile([128, NH], BF, tag="wv")
                nc.sync.dma_start(out=wv_t[:], in_=wv_up[j])
                wv_tiles.append(wv_t)
            for h in range(HP):
                for j in range(4):
                    nc.tensor.matmul(
                        ps_vn[0:128, h * B:(h + 1) * B],
                        wv_tiles[j][:, h * D:(h + 1) * D],
                        ckvT[:, j * B:(j + 1) * B],
                        start=(j == 0), stop=(j == 3),
                    )

            # ---------------- phase A: scoresT [128 k, 4j * 128 hb] ----------------
            # kt tile g holds hb=4g..4g+3 as [128 d, (u, l)]; stationary slice
            # [128 d, 128 k] per (hb, j), moving = qT column hb.
            ps_sc = psB.tile([128, 512], F32, tag="bank", name="ps_sc")
            for g in range(32):
                kt_t = ktp.tile([128, 2048], KVDT, tag="kt")
                nc.sync.dma_start(out=kt_t[:], in_=kt[g])
                for u in range(4):
                    hb = 4 * g + u
                    for j in range(4):
                        nc.tensor.matmul(
                            ps_sc[0:128, j * 128 + hb:j * 128 + hb + 1],
                            kt_t[:, u * 512 + j * 128:u * 512 + (j + 1) * 128],
                            qT[:, hb:hb + 1],
                            start=True, stop=True,
                        )

            # softmax pieces: probsT = exp(EXP_SCALE * scoresT) (unnormalized),
            # denom via ones-matmul, reciprocal broadcast to [128, hb].
            # bias shifts exp into fp8 range (max |score*scale| ~ 6); softmax
            # is shift-invariant so the denominator cancels it exactly.
            probsT = sb.tile([128, 512], KVDT, tag="probsT")
            exp_bias = constp.tile([128, 1], F32, tag="exp_bias")
            nc.vector.memset(exp_bias[:], -2.5)
            nc.scalar.activation(
                probsT[:], ps_sc[0:128, 0:512],
                mybir.ActivationFunctionType.Exp, scale=EXP_SCALE, bias=exp_bias[:],
            )
            ones_kv = constp.tile([128, 1], KVDT, tag="ones_kv")
            nc.vector.memset(ones_kv[:], 1.0)
            ps_dn = psB.tile([1, 128], F32, tag="bank", name="ps_dn")
            for j in range(4):
                nc.tensor.matmul(
                    ps_dn[0:1, 0:128],
                    ones_kv[:],
                    probsT[:, j * 128:(j + 1) * 128],
                    start=(j == 0), stop=(j == 3),
                )
            recip = sb.tile([1, 128], F32, tag="recip")
            nc.vector.reciprocal(recip[:], ps_dn[0:1, 0:128])
            ones_f32 = constp.tile([1, 128], F32, tag="ones_f32")
            nc.vector.memset(ones_f32[:], 1.0)
            ps_rb = psB.tile([128, 128], F32, tag="bank", name="ps_rb")
            nc.tensor.matmul(
                ps_rb[0:128, 0:128], ones_f32[:], recip[:], start=True, stop=True,
            )
            rb_sb = sb.tile([128, 128], F32, tag="rb_sb")
            nc.vector.tensor_copy(rb_sb[:], ps_rb[0:128, 0:128])

            # ---------------- phase B: cacheT [128 d, 128 hb] ----------------
            # v tile g holds hb=4g..4g+3 as [128 kk, (u, j, d)]; stationary
            # slice [128 kk, 128 d] per (hb, j), moving = probsT column.
            # attnT combine happens per head right after its two v tiles, so
            # phase C's h-loop can chase phase B head by head.
            ps_ca = psA.tile([128, 128], F32, tag="bank", name="ps_ca")
            tmp_at = sb.tile([128, 128], F32, tag="tmp_at")
            attnT = sb.tile([128, 128], BF, tag="attnT")
            for g in range(32):
                v_t = vp.tile([128, 2048], KVDT, tag="v")
                nc.sync.dma_start(out=v_t[:], in_=v[g])
                for u in range(4):
                    hb = 4 * g + u
                    for j in range(4):
                        nc.tensor.matmul(
                            ps_ca[0:128, hb:hb + 1],
                            v_t[:, (u * 4 + j) * 128:(u * 4 + j + 1) * 128],
                            probsT[:, j * 128 + hb:j * 128 + hb + 1],
                            start=(j == 0), stop=(j == 3),
                        )
                if g % 2 == 1:
                    # head h = g//2 complete: attnT_h = cacheT_h/denom + v_newT_h
                    hc = g // 2
                    cs = slice(hc * B, (hc + 1) * B)
                    nc.vector.tensor_mul(
                        tmp_at[:, cs], ps_ca[0:128, cs], rb_sb[:, cs]
                    )
                    nc.vector.tensor_add(
                        attnT[:, cs], tmp_at[:, cs], ps_vn[0:128, cs]
                    )

            # ---------------- phase C: o_part = attnT.T @ Wo_c ----------------
            # Output cols split in two halves; each half's ReduceScatter
            # launches as soon as its rounds finish so the collective overlaps
            # the remaining compute.
            o_bounce0 = dram.tile([B, 12 * 512], F32, tag="ob0")
            o_bounce1 = dram.tile([B, HID - 12 * 512], F32, tag="ob1")
            for n0, n1 in ((0, 4), (4, 8), (8, 12), (12, 14)):
                nn = n1 - n0
                ps_os = [
                    psB.tile([8, 512], F32, tag="bank", name=f"ps_o{n0}_{i}")
                    for i in range(nn)
                ]
                for h in range(HP):
                    wo_t = wop.tile([128, 2048], BF, tag="wo")
                    nc.sync.dma_start(
                        out=wo_t[:, 0:nn * 512],
                        in_=wo[h * D:(h + 1) * D, n0 * 512:n1 * 512],
                    )
                    for i in range(nn):
                        nc.tensor.matmul(
                            ps_os[i][:8, :],
                            attnT[:, h * B:(h + 1) * B],
                            wo_t[:, i * 512:(i + 1) * 512],
                            start=(h == 0), stop=(h == HP - 1),
                        )
                for i in range(nn):
                    ostage = stg.tile([8, 512], F32, tag="ostage")
                    nc.vector.tensor_copy(ostage[:], ps_os[i][:8, :])
                    if n0 + i < 12:
                        nc.sync.dma_start(
                            out=o_bounce0[:, (n0 + i) * 512:(n0 + i + 1) * 512],
                            in_=ostage[:],
                        )
                    else:
                        nc.sync.dma_start(
                            out=o_bounce1[:, (n0 + i - 12) * 512:(n0 + i - 11) * 512],
                            in_=ostage[:],
                        )
                if n1 == 12:
                    o_rs0 = dram.tile([1, 12 * 512], F32, tag="ors0")
                    nc.gpsimd.collective_compute(
                        "ReduceScatter",
                        mybir.AluOpType.add,
                        replica_groups=rg,
                        ins=[o_bounce0.opt()],
                        outs=[o_rs0.opt()],
                    )
                    nc.sync.dma_start(out=o[:, 0:12 * 512], in_=o_rs0[:])

            o_rs1 = dram.tile([1, HID - 12 * 512], F32, tag="ors1")
            nc.gpsimd.collective_compute(
                "ReduceScatter",
                mybir.AluOpType.add,
                replica_groups=rg,
                ins=[o_bounce1.opt()],
                outs=[o_rs1.opt()],
            )
            nc.sync.dma_start(out=o[:, 12 * 512:], in_=o_rs1[:])

    nc.compile()
    return nc


_NC_CACHE = None


def _get_nc():
    global _NC_CACHE
    if _NC_CACHE is None:
        _NC_CACHE = build_nc()
    return _NC_CACHE


def make_in_maps(x, k_cache, v_cache, Wq_down, Wq_up, Wkv_down, Wv_up, Wo):
    x = np.asarray(x, np.float32).reshape(B, HID)
    k_cache = np.asarray(k_cache, np.float32)
    v_cache = np.asarray(v_cache, np.float32)
    Wq_down = np.asarray(Wq_down, np.float32)
    Wq_up = np.asarray(Wq_up, np.float32)
    Wkv_down = np.asarray(Wkv_down, np.float32)
    Wv_up = np.asarray(Wv_up, np.float32)
    Wo = np.asarray(Wo, np.float32)

    np_kv = NP_FP8 if USE_FP8_KV else NP_BF16
    np_wq = NP_FP8 if USE_FP8_WQ else NP_BF16

    # xt[p, i*8+b] = x[b, i*128+p]
    xt = np.ascontiguousarray(
        x.T.reshape(56, 128, B).transpose(1, 0, 2).reshape(128, 56 * B)
    ).astype(NP_BF16)
    Wcat = np.concatenate([Wq_down, Wkv_down], axis=1)  # [7168, 2048]

    in_maps = []
    for c in range(NC_):
        hs = slice(c * HP, (c + 1) * HP)
        # w_down[p, i*COLP + n] = Wcat[i*128+p, c*COLP+n]
        wd = np.ascontiguousarray(
            Wcat[:, c * COLP:(c + 1) * COLP]
            .reshape(56, 128, COLP).transpose(1, 0, 2).reshape(128, 56 * COLP)
        ).astype(NP_BF16)
        wq = np.ascontiguousarray(
            (Wq_up[:, c * NH:(c + 1) * NH] * WQ_SCALE).reshape(12, 128, NH)
        ).astype(np_wq)
        wv = np.ascontiguousarray(
            Wv_up[:, c * NH:(c + 1) * NH].reshape(4, 128, NH)
        ).astype(NP_BF16)
        wo_c = np.ascontiguousarray(Wo[c * NH:(c + 1) * NH, :]).astype(NP_BF16)
        # kt tile g: [128 d, (u, l)] for hb=4g+u
        kt_c = np.ascontiguousarray(
            k_cache[:, hs]
            .transpose(1, 0, 3, 2)          # [h, b, d, l] (16, 8, 128, 512)
            .reshape(32, 4, 128, 512)       # [g, u, d, l]
            .transpose(0, 2, 1, 3)          # [g, d, u, l]
            .reshape(32, 128, 2048)
        ).astype(np_kv)
        # v tile g: [128 kk, (u, j, d)] for hb=4g+u, l = j*128+kk
        v_c = np.ascontiguousarray(
            v_cache[:, hs]
            .transpose(1, 0, 2, 3)          # [h, b, l, d] (16, 8, 512, 128)
            .reshape(32, 4, 4, 128, 128)    # [g, u, j, kk, d]
            .transpose(0, 3, 1, 2, 4)       # [g, kk, u, j, d]
            .reshape(32, 128, 2048)
        ).astype(np_kv)
        in_maps.append(
            {
                "xt": xt,
                "w_down": wd,
                "wq_up": wq,
                "wv_up": wv,
                "kt": kt_c,
                "v": v_c,
                "wo": wo_c,
            }
        )
    return in_maps


def kernel(x, k_cache, v_cache, Wq_down, Wq_up, Wkv_down, Wk_up, Wv_up, Wo, **_):
    in_maps = make_in_maps(
        x, k_cache, v_cache, Wq_down, Wq_up, Wkv_down, Wv_up, Wo
    )
    nc = _get_nc()
    res = bass_utils.run_bass_kernel_spmd(nc, in_maps, core_ids=list(range(NC_)))
    out = np.stack([res.results[b]["o"] for b in range(B)], axis=0)  # (8, 1, 7168)
    return np.ascontiguousarray(out, dtype=np.float32)
